# revision 18
# baseline (speedup 1.0000x reference)
"""AGRU cell (antisymmetric GRU) forward on 8 TRN2 NeuronCores.

Data-parallel: batch 16384 is sharded 2048 rows/core; the six 1024x1024
weight matrices are replicated. No cross-core communication.

Everything on-device is computed in "hidden-major" (transposed) layout:
    zT = sigmoid(Wz @ xT + Uz @ hT + bz)        [H, B]
    rT = sigmoid(Wr @ xT + Ur @ hT + br)
    rhT = rT * hT
    dhT = tanh(Vh @ xT + A @ rhT + bh)
    outT = hT + eps * zT * dhT
so every matmul has the (pre-transposed, host-prepared) weight tile as the
stationary operand and xT/hT/rhT as the moving operand, and nothing ever
needs an on-device transpose.  The host transposes each core's [1024, 2048]
result back when assembling the full output.

Precision: ALL six GEMMs run in fp8 e4m3 with DoubleRow perf mode (2
contraction rows/cycle, the fp8 peak); inputs are pre-scaled (x*16, W*256)
and the scale is folded back in the activation (scale=1/4096), which also
applies the per-partition bias.  All accumulation is fp32 in PSUM; the
residual h + eps*z*dh reads h in bf16 and runs fp32 on the vector engine.
Measured rel err vs the fp32 reference: 1.59e-2 (gate: 2e-2).

Schedule: the PE is the bottleneck (768 DR matmuls x 216ns = 166us fp8
roofline), so everything is built to keep it saturated:
  - DMA queues process ~1 descriptor / ~1.7us at startup regardless of
    size, so startup-critical tensors are host-packed into ~512KB
    descriptors: x8+h8 fused into one [2, 128, 2, KC, 1024] tensor
    (k-pair x both-operands blocks, 1KB contiguous rows), the r-gate
    weight pair per jt in one block, the four phase-2 weights per jt in
    one block.
  - bulk data on the sync HWDGE queue (earliest completion signaling),
    all weights on gpsimd; the first matmul group consumes x/h k-pairs
    alternately, exactly the order descriptors land.
  - junk warm-up matmuls ramp the PE clock p-state through the initial
    DMA window; junk activations pull the ACT table load forward.
  - phase 2 interleaves the z-GEMM and dh-GEMM per 512-column block; the
    final block tapers (256/128/128) so only a short tanh+mul+residual+
    DMA chain trails the very last matmul.
"""

import sys

sys.path.insert(0, "/opt/trn_rl_repo")

import numpy as np
import ml_dtypes

from contextlib import ExitStack

import concourse.bass as bass
import concourse.mybir as mybir
from concourse import bacc, tile
from concourse.bass import ds, ts
from concourse.bass_utils import run_bass_kernel_spmd

BF16 = mybir.dt.bfloat16
FP8 = mybir.dt.float8e4
F32 = mybir.dt.float32
AFT = mybir.ActivationFunctionType
ALU = mybir.AluOpType
DR = mybir.MatmulPerfMode.DoubleRow

# fp8 pre-scaling for all GEMMs: data*16, weights*256, compensated by
# activation scale 1/(16*256).
SCALE_X = 16.0
SCALE_W = 256.0
INV_SCALE = 1.0 / (SCALE_X * SCALE_W)

N_CORES = 8
BATCH = 16384
B = BATCH // N_CORES  # per-core batch shard (2048)
H = 1024  # hidden == input size
KC = H // 128  # contraction chunks (8)
JT = H // 128  # output row tiles (8)
NB = 4  # moving-dim (batch) blocks per psum bank
NBS = B // NB  # 512 columns per matmul
DB = 2  # DMA column-blocks of x8/h8 (1024 cols -> 1KB fp8 rows)
DBS = B // DB
GAMMA = 0.01

_nc_cache = {}


def _build(eps: float):
    """Build + compile the single-core Tile program (same graph on all cores)."""
    nc = bacc.Bacc("TRN2", target_bir_lowering=False, debug=False)

    hT_d = nc.dram_tensor("hT", [128, KC, B], BF16, kind="ExternalInput")
    # x8 and h8 fused: one descriptor carries a k-chunk-pair of BOTH
    # operands for 1024 columns (512KB, 1KB rows). The (k, t) pair is
    # pre-interleaved as one stride-uniform dim of 4 so the DMA access
    # patterns stay <= 3 dims after merging.
    xh8_d = nc.dram_tensor("xh8", [DB, KC // 2, 128, 4, DBS], FP8, kind="ExternalInput")
    # r-gate weights paired per jt: [jt, part, t(wr/ur), kc, col]
    rw8_d = nc.dram_tensor("rw8", [JT, 128, 2, KC, 128], FP8, kind="ExternalInput")
    # phase-2 weights quadded per jt: t = (wz, uz, vh, at)
    p2w8_d = nc.dram_tensor("p2w8", [JT, 128, 4, KC, 128], FP8, kind="ExternalInput")
    bias_d = nc.dram_tensor("biases", [128, 24], F32, kind="ExternalInput")
    out_d = nc.dram_tensor("out", [H, B], F32, kind="ExternalOutput")

    with tile.TileContext(nc) as tc, ExitStack() as ctx:
        singles = ctx.enter_context(tc.tile_pool(name="singles", bufs=1))
        wpool = ctx.enter_context(tc.tile_pool(name="wpool", bufs=8))
        # phase-2 weight quads: bufs=2 throttles the prefetch so these 4MB
        # don't contend with the startup-critical streams for HBM fabric
        p2wp = ctx.enter_context(tc.tile_pool(name="p2wp", bufs=2))
        psum = ctx.enter_context(tc.tile_pool(name="psum", bufs=8, space="PSUM"))
        workp = ctx.enter_context(tc.tile_pool(name="workp", bufs=8))

        hTb = singles.tile([128, KC, B], BF16)
        xh8 = singles.tile([128, KC, 2, B], FP8)
        rhT8 = singles.tile([128, KC, B], FP8)
        bias_sb = singles.tile([128, 24], F32)
        wjunk = singles.tile([128, 2, 128], FP8)
        xjunk = singles.tile([128, 2, NBS], FP8)

        # ---- PE warm-up: junk matmuls ramp the clock p-state and keep  ----
        # ---- the PE busy through the first-DMA completion-semaphore    ----
        # ---- window; junk activations pull the ACT table load forward. ----
        nc.vector.memset(wjunk[:], 0.0)
        nc.vector.memset(xjunk[:], 0.0)
        psj = psum.tile([128, NBS], F32, tag="ps", name="ps_warm")
        for i in range(8):
            nc.tensor.matmul(
                psj[:], wjunk[:], xjunk[:], start=(i == 0), stop=(i == 6),
                perf_mode=DR,
            )
        aj = workp.tile([128, 2, 128], BF16, tag="actwarm")
        nc.scalar.activation(aj[:, 0, :], wjunk[:, 0, :], AFT.Sigmoid)
        nc.scalar.activation(aj[:, 1, :], wjunk[:, 1, :], AFT.Tanh)

        # DMA issue order == consumption order.
        #   gpsimd: r-weight pair per jt (jt0 first), then phase-2 quads
        #   sync:   xh8 block0 by k-pair, bias, xh8 block1, bf16 h, out
        rw = []
        for jt in range(JT):
            w = wpool.tile([128, 2, KC, 128], FP8, tag="rw")
            if jt == 0:
                # split jt0's pair: the first LDWEIGHTS only needs wr0
                nc.gpsimd.dma_start(out=w[:, 0], in_=rw8_d[0][:, 0])
                nc.gpsimd.dma_start(out=w[:, 1], in_=rw8_d[0][:, 1])
            else:
                nc.gpsimd.dma_start(out=w[:], in_=rw8_d[jt])
            rw.append(w)
        for kcp in range(KC // 2):
            nc.sync.dma_start(
                out=xh8[:, 2 * kcp : 2 * kcp + 2, :, ds(0, DBS)],
                in_=xh8_d[0][kcp],
            )
        nc.sync.dma_start(out=bias_sb[:], in_=bias_d[:])
        for kcp in range(KC // 2):
            nc.sync.dma_start(
                out=xh8[:, 2 * kcp : 2 * kcp + 2, :, ds(DBS, DBS)],
                in_=xh8_d[1][kcp],
            )
        for c in range(KC):
            nc.sync.dma_start(out=hTb[:, c, :], in_=hT_d[:, c, :])

        def xh_slice(rhs, k, col0, ncol):
            # rhs is either (xh8, t) into the fused tile or the rhT8 tile
            if isinstance(rhs, tuple):
                t8, tsel = rhs
                return t8[:, k : k + 2, tsel, ds(col0, ncol)]
            return rhs[:, k : k + 2, ds(col0, ncol)]

        def gemm_pair(ps, wA, rhsA, wB, rhsB, col0, ncol, interleave=False):
            # one PSUM accumulation group: A@a + B@b, fp8 DoubleRow.
            # interleaved k-order consumes one x and one h k-pair
            # alternately (startup: matches descriptor arrival order).
            mms = []
            for k in range(0, KC, 2):
                mms.append((wA, rhsA, k))
                if interleave:
                    mms.append((wB, rhsB, k))
            if not interleave:
                for k in range(0, KC, 2):
                    mms.append((wB, rhsB, k))
            for i, (w, rhs, k) in enumerate(mms):
                nc.tensor.matmul(
                    ps[:, ds(0, ncol)],
                    w[:, k : k + 2, :],
                    xh_slice(rhs, k, col0, ncol),
                    start=(i == 0),
                    stop=(i == len(mms) - 1),
                    perf_mode=DR,
                )


        # ---- phase 1: r gate (hidden-major, fp8), rhT = sigmoid(...) * hT ----
        # Batch-block-outer, jt-inner: the first pass needs only the first
        # 1024-column xh8 block plus the r weights, so the PE starts early
        # and never waits on the bulk DMA. The 8 jt groups of one pass
        # exactly fill the 8 PSUM banks.
        for nb in range(NB):
            for jt in range(JT):
                w = rw[jt]
                ps = psum.tile([128, NBS], F32, tag="ps", name=f"ps_r{jt}_{nb}")
                gemm_pair(
                    ps, w[:, 0], (xh8, 0), w[:, 1], (xh8, 1),
                    nb * NBS, NBS, interleave=(nb == 0),
                )
                rt = workp.tile([128, NBS], BF16, tag="act")
                nc.scalar.activation(
                    rt[:],
                    ps[:],
                    AFT.Sigmoid,
                    bias=bias_sb[:, 8 + jt : 9 + jt],
                    scale=INV_SCALE,
                )
                # rh in scaled fp8 for the DoubleRow A-matmul: r * (h*16).
                nc.vector.tensor_mul(
                    rhT8[:, jt, ds(nb * NBS, NBS)],
                    rt[:],
                    xh8[:, jt, 1, ds(nb * NBS, NBS)],
                )

        # ---- phase 2: z gate + delta_h + residual, all fp8, per (jt, nb) ----
        # z-GEMM and dh-GEMM interleave per column block, so each PSUM bank
        # is evicted while the next accumulates. The final block runs as
        # 256/128/128 tapers so only a short chain trails the last matmul.
        for jt in range(JT):
            pw = p2wp.tile([128, 4, KC, 128], FP8, tag="w")
            nc.gpsimd.dma_start(out=pw[:], in_=p2w8_d[jt])
            last = jt == JT - 1
            blocks = [(nb * NBS, NBS) for nb in range(NB - 1 if last else NB)]
            if last:
                b0 = (NB - 1) * NBS
                blocks += [(b0, 256), (b0 + 256, 128), (b0 + 384, 64), (b0 + 448, 64)]
            for bi, (col0, ncol) in enumerate(blocks):
                psz = psum.tile([128, NBS], F32, tag="ps", name=f"ps_z{jt}_{bi}")
                gemm_pair(psz, pw[:, 0], (xh8, 0), pw[:, 1], (xh8, 1), col0, ncol)
                psd = psum.tile([128, NBS], F32, tag="ps", name=f"ps_d{jt}_{bi}")
                gemm_pair(psd, pw[:, 2], (xh8, 0), pw[:, 3], rhT8, col0, ncol)
                zt = workp.tile([128, NBS], BF16, tag="act")
                nc.scalar.activation(
                    zt[:, ds(0, ncol)],
                    psz[:, ds(0, ncol)],
                    AFT.Sigmoid,
                    bias=bias_sb[:, jt : jt + 1],
                    scale=INV_SCALE,
                )
                dt_ = workp.tile([128, NBS], BF16, tag="act")
                nc.scalar.activation(
                    dt_[:, ds(0, ncol)],
                    psd[:, ds(0, ncol)],
                    AFT.Tanh,
                    bias=bias_sb[:, 16 + jt : 17 + jt],
                    scale=INV_SCALE,
                )
                zdh = workp.tile([128, NBS], F32, tag="zdh")
                nc.vector.tensor_mul(
                    zdh[:, ds(0, ncol)], zt[:, ds(0, ncol)], dt_[:, ds(0, ncol)]
                )
                # out = (z*dh) * eps + h
                ot = workp.tile([128, NBS], F32, tag="out")
                nc.vector.scalar_tensor_tensor(
                    ot[:, ds(0, ncol)],
                    zdh[:, ds(0, ncol)],
                    float(eps),
                    hTb[:, jt, ds(col0, ncol)],
                    op0=ALU.mult,
                    op1=ALU.add,
                )
                nc.sync.dma_start(
                    out=out_d[ts(jt, 128), ds(col0, ncol)],
                    in_=ot[:, ds(0, ncol)],
                )

    nc.compile()
    return nc


def _get_nc(eps: float):
    key = float(eps)
    if key not in _nc_cache:
        _nc_cache[key] = _build(key)
    return _nc_cache[key]


def _block_weight(wT, dtype, scale=1.0):
    # [1024, 1024] (contraction-major) -> [jt, p, c, j] st. blk[jt,p,c,j] = wT[c*128+p, jt*128+j]
    blk = wT.reshape(KC, 128, JT, 128).transpose(2, 1, 0, 3)
    if scale != 1.0:
        blk = blk * scale
    return np.ascontiguousarray(blk).astype(dtype)


def _block_data(m):
    # per-core [B, 1024] -> [p, c, b] st. blk[p,c,b] = m[b, c*128+p]
    return np.ascontiguousarray(m.T.reshape(KC, 128, B).transpose(1, 0, 2))


def _block_data_db(m):
    # per-core [B, 1024] -> [db, p, c, dbs] column-block-major (1KB rows)
    blk = m.T.reshape(KC, 128, DB, DBS).transpose(2, 1, 0, 3)
    return np.ascontiguousarray(blk)


def _prep_shared(W_z, b_z, U_z, W_r, b_r, U_r, V_h, b_h, W_h):
    F8 = ml_dtypes.float8_e4m3
    A = W_h - W_h.T - GAMMA * np.eye(H, dtype=np.float32)
    blk = lambda w: _block_weight(w.T, F8, SCALE_W)
    # [jt, p, t, kc, 128] packings
    rw8 = np.ascontiguousarray(
        np.stack([blk(W_r), blk(U_r)], axis=2)
    )
    p2w8 = np.ascontiguousarray(
        np.stack([blk(W_z), blk(U_z), blk(V_h), blk(A)], axis=2)
    )
    biases = np.ascontiguousarray(
        np.concatenate(
            [
                b_z.reshape(JT, 128).T,
                b_r.reshape(JT, 128).T,
                b_h.reshape(JT, 128).T,
            ],
            axis=1,
        ).astype(np.float32)
    )
    return {"rw8": rw8, "p2w8": p2w8, "biases": biases}


def _prep_in_maps(x, h_prev, W_z, b_z, U_z, W_r, b_r, U_r, V_h, b_h, W_h):
    BF = ml_dtypes.bfloat16
    F8 = ml_dtypes.float8_e4m3
    h16 = np.asarray(h_prev, np.float32).astype(BF)
    x8 = (np.asarray(x, np.float32) * SCALE_X).astype(F8)
    h8 = (np.asarray(h_prev, np.float32) * SCALE_X).astype(F8)

    shared = _prep_shared(W_z, b_z, U_z, W_r, b_r, U_r, V_h, b_h, W_h)
    in_maps = []
    for c in range(N_CORES):
        sl = slice(c * B, (c + 1) * B)
        # fused [db, kcp, p, (dk,t), dbs]
        st = np.stack([_block_data_db(x8[sl]), _block_data_db(h8[sl])], axis=3)
        st = st.reshape(DB, 128, KC // 2, 2, 2, DBS).transpose(0, 2, 1, 3, 4, 5)
        xh8 = np.ascontiguousarray(st.reshape(DB, KC // 2, 128, 4, DBS))
        in_maps.append({"hT": _block_data(h16[sl]), "xh8": xh8, **shared})
    return in_maps


def run(inputs, trace=False):
    """Returns (full_output [16384,1024] f32, BassKernelResults)."""
    np_in = {k: np.asarray(v, np.float32) for k, v in inputs.items()}
    eps = float(np_in.pop("epsilon"))
    in_maps = _prep_in_maps(**np_in)
    nc = _get_nc(eps)
    res = run_bass_kernel_spmd(
        nc, in_maps, core_ids=list(range(N_CORES)), trace=trace
    )
    out = np.empty((BATCH, H), np.float32)
    for c in range(N_CORES):
        out[c * B : (c + 1) * B, :] = res.results[c]["out"].T
    return out, res


def kernel(**inputs) -> np.ndarray:
    out, _ = run(inputs, trace=False)
    return out


# revision 19
# speedup vs baseline: 1.0046x; 1.0046x over previous
"""AGRU cell (antisymmetric GRU) forward on 8 TRN2 NeuronCores.

Data-parallel: batch 16384 is sharded 2048 rows/core; the six 1024x1024
weight matrices are replicated. No cross-core communication.

Everything on-device is computed in "hidden-major" (transposed) layout:
    zT = sigmoid(Wz @ xT + Uz @ hT + bz)        [H, B]
    rT = sigmoid(Wr @ xT + Ur @ hT + br)
    rhT = rT * hT
    dhT = tanh(Vh @ xT + A @ rhT + bh)
    outT = hT + eps * zT * dhT
so every matmul has the (pre-transposed, host-prepared) weight tile as the
stationary operand and xT/hT/rhT as the moving operand, and nothing ever
needs an on-device transpose.  The host transposes each core's [1024, 2048]
result back when assembling the full output.

Precision: ALL six GEMMs run in fp8 e4m3 with DoubleRow perf mode (2
contraction rows/cycle, the fp8 peak); inputs are pre-scaled (x*16, W*256)
and the scale is folded back in the activation (scale=1/4096), which also
applies the per-partition bias.  All accumulation is fp32 in PSUM; the
residual h + eps*z*dh reads h in bf16 and runs fp32 on the vector engine.
Measured rel err vs the fp32 reference: 1.59e-2 (gate: 2e-2).

Schedule: the PE is the bottleneck (768 DR matmuls x 216ns = 166us fp8
roofline), so everything is built to keep it saturated:
  - DMA queues process ~1 descriptor / ~1.7us at startup regardless of
    size, so startup-critical tensors are host-packed into ~512KB
    descriptors: x8+h8 fused into one [2, 128, 2, KC, 1024] tensor
    (k-pair x both-operands blocks, 1KB contiguous rows), the r-gate
    weight pair per jt in one block, the four phase-2 weights per jt in
    one block.
  - bulk data on the sync HWDGE queue (earliest completion signaling),
    all weights on gpsimd; the first matmul group consumes x/h k-pairs
    alternately, exactly the order descriptors land.
  - junk warm-up matmuls ramp the PE clock p-state through the initial
    DMA window; junk activations pull the ACT table load forward.
  - phase 2 interleaves the z-GEMM and dh-GEMM per 512-column block; the
    final block tapers (256/128/128) so only a short tanh+mul+residual+
    DMA chain trails the very last matmul.
"""

import sys

sys.path.insert(0, "/opt/trn_rl_repo")

import numpy as np
import ml_dtypes

from contextlib import ExitStack

import concourse.bass as bass
import concourse.mybir as mybir
from concourse import bacc, tile
from concourse.bass import ds, ts
from concourse.bass_utils import run_bass_kernel_spmd

BF16 = mybir.dt.bfloat16
FP8 = mybir.dt.float8e4
F32 = mybir.dt.float32
AFT = mybir.ActivationFunctionType
ALU = mybir.AluOpType
DR = mybir.MatmulPerfMode.DoubleRow

# fp8 pre-scaling for all GEMMs: data*16, weights*256, compensated by
# activation scale 1/(16*256).
SCALE_X = 16.0
SCALE_W = 256.0
INV_SCALE = 1.0 / (SCALE_X * SCALE_W)

N_CORES = 8
BATCH = 16384
B = BATCH // N_CORES  # per-core batch shard (2048)
H = 1024  # hidden == input size
KC = H // 128  # contraction chunks (8)
JT = H // 128  # output row tiles (8)
NB = 4  # moving-dim (batch) blocks per psum bank
NBS = B // NB  # 512 columns per matmul
DB = 2  # DMA column-blocks of x8/h8 (1024 cols -> 1KB fp8 rows)
DBS = B // DB
GAMMA = 0.01

_nc_cache = {}


def _build(eps: float):
    """Build + compile the single-core Tile program (same graph on all cores)."""
    nc = bacc.Bacc("TRN2", target_bir_lowering=False, debug=False)

    hT_d = nc.dram_tensor("hT", [128, KC, B], BF16, kind="ExternalInput")
    # x8 and h8 fused: one descriptor carries a k-chunk-pair of BOTH
    # operands for 1024 columns (512KB, 1KB rows). The (k, t) pair is
    # pre-interleaved as one stride-uniform dim of 4 so the DMA access
    # patterns stay <= 3 dims after merging.
    xh8_d = nc.dram_tensor("xh8", [DB, KC // 2, 128, 4, DBS], FP8, kind="ExternalInput")
    # r-gate weights paired per jt: [jt, part, t(wr/ur), kc, col]
    rw8_d = nc.dram_tensor("rw8", [JT, 128, 2, KC, 128], FP8, kind="ExternalInput")
    # phase-2 weights quadded per jt: t = (wz, uz, vh, at)
    p2w8_d = nc.dram_tensor("p2w8", [JT, 128, 4, KC, 128], FP8, kind="ExternalInput")
    bias_d = nc.dram_tensor("biases", [128, 24], F32, kind="ExternalInput")
    out_d = nc.dram_tensor("out", [H, B], F32, kind="ExternalOutput")

    with tile.TileContext(nc) as tc, ExitStack() as ctx:
        singles = ctx.enter_context(tc.tile_pool(name="singles", bufs=1))
        wpool = ctx.enter_context(tc.tile_pool(name="wpool", bufs=8))
        # phase-2 weight quads: bufs=2 throttles the prefetch so these 4MB
        # don't contend with the startup-critical streams for HBM fabric
        p2wp = ctx.enter_context(tc.tile_pool(name="p2wp", bufs=2))
        psum = ctx.enter_context(tc.tile_pool(name="psum", bufs=8, space="PSUM"))
        workp = ctx.enter_context(tc.tile_pool(name="workp", bufs=8))

        hTb = singles.tile([128, KC, B], BF16)
        xh8 = singles.tile([128, KC, 2, B], FP8)
        rhT8 = singles.tile([128, KC, B], FP8)
        bias_sb = singles.tile([128, 24], F32)
        wjunk = singles.tile([128, 2, 128], FP8)
        xjunk = singles.tile([128, 2, NBS], FP8)

        # ---- PE warm-up: junk matmuls ramp the clock p-state and keep  ----
        # ---- the PE busy through the first-DMA completion-semaphore    ----
        # ---- window; junk activations pull the ACT table load forward. ----
        nc.vector.memset(wjunk[:], 0.0)
        nc.vector.memset(xjunk[:], 0.0)
        psj = psum.tile([128, NBS], F32, tag="ps", name="ps_warm")
        for i in range(7):
            nc.tensor.matmul(
                psj[:], wjunk[:], xjunk[:], start=(i == 0), stop=(i == 6),
                perf_mode=DR,
            )
        aj = workp.tile([128, 2, 128], BF16, tag="actwarm")
        nc.scalar.activation(aj[:, 0, :], wjunk[:, 0, :], AFT.Sigmoid)
        nc.scalar.activation(aj[:, 1, :], wjunk[:, 1, :], AFT.Tanh)

        # DMA issue order == consumption order.
        #   gpsimd: r-weight pair per jt (jt0 first), then phase-2 quads
        #   sync:   xh8 block0 by k-pair, bias, xh8 block1, bf16 h, out
        rw = []
        for jt in range(JT):
            w = wpool.tile([128, 2, KC, 128], FP8, tag="rw")
            if jt == 0:
                # split jt0's pair: the first LDWEIGHTS only needs wr0
                nc.gpsimd.dma_start(out=w[:, 0], in_=rw8_d[0][:, 0])
                nc.gpsimd.dma_start(out=w[:, 1], in_=rw8_d[0][:, 1])
            else:
                nc.gpsimd.dma_start(out=w[:], in_=rw8_d[jt])
            rw.append(w)
        for kcp in range(KC // 2):
            nc.sync.dma_start(
                out=xh8[:, 2 * kcp : 2 * kcp + 2, :, ds(0, DBS)],
                in_=xh8_d[0][kcp],
            )
        nc.sync.dma_start(out=bias_sb[:], in_=bias_d[:])
        for kcp in range(KC // 2):
            nc.sync.dma_start(
                out=xh8[:, 2 * kcp : 2 * kcp + 2, :, ds(DBS, DBS)],
                in_=xh8_d[1][kcp],
            )
        for c in range(KC):
            nc.sync.dma_start(out=hTb[:, c, :], in_=hT_d[:, c, :])

        def xh_slice(rhs, k, col0, ncol):
            # rhs is either (xh8, t) into the fused tile or the rhT8 tile
            if isinstance(rhs, tuple):
                t8, tsel = rhs
                return t8[:, k : k + 2, tsel, ds(col0, ncol)]
            return rhs[:, k : k + 2, ds(col0, ncol)]

        def gemm_pair(ps, wA, rhsA, wB, rhsB, col0, ncol, interleave=False):
            # one PSUM accumulation group: A@a + B@b, fp8 DoubleRow.
            # interleaved k-order consumes one x and one h k-pair
            # alternately (startup: matches descriptor arrival order).
            mms = []
            for k in range(0, KC, 2):
                mms.append((wA, rhsA, k))
                if interleave:
                    mms.append((wB, rhsB, k))
            if not interleave:
                for k in range(0, KC, 2):
                    mms.append((wB, rhsB, k))
            for i, (w, rhs, k) in enumerate(mms):
                nc.tensor.matmul(
                    ps[:, ds(0, ncol)],
                    w[:, k : k + 2, :],
                    xh_slice(rhs, k, col0, ncol),
                    start=(i == 0),
                    stop=(i == len(mms) - 1),
                    perf_mode=DR,
                )


        # ---- phase 1: r gate (hidden-major, fp8), rhT = sigmoid(...) * hT ----
        # Batch-block-outer, jt-inner: the first pass needs only the first
        # 1024-column xh8 block plus the r weights, so the PE starts early
        # and never waits on the bulk DMA. The 8 jt groups of one pass
        # exactly fill the 8 PSUM banks.
        for nb in range(NB):
            for jt in range(JT):
                w = rw[jt]
                ps = psum.tile([128, NBS], F32, tag="ps", name=f"ps_r{jt}_{nb}")
                gemm_pair(
                    ps, w[:, 0], (xh8, 0), w[:, 1], (xh8, 1),
                    nb * NBS, NBS, interleave=(nb == 0),
                )
                rt = workp.tile([128, NBS], BF16, tag="act")
                nc.scalar.activation(
                    rt[:],
                    ps[:],
                    AFT.Sigmoid,
                    bias=bias_sb[:, 8 + jt : 9 + jt],
                    scale=INV_SCALE,
                )
                # rh in scaled fp8 for the DoubleRow A-matmul: r * (h*16).
                nc.vector.tensor_mul(
                    rhT8[:, jt, ds(nb * NBS, NBS)],
                    rt[:],
                    xh8[:, jt, 1, ds(nb * NBS, NBS)],
                )

        # ---- phase 2: z gate + delta_h + residual, all fp8, per (jt, nb) ----
        # z-GEMM and dh-GEMM interleave per column block, so each PSUM bank
        # is evicted while the next accumulates. The final block runs as
        # 256/128/128 tapers so only a short chain trails the last matmul.
        for jt in range(JT):
            pw = p2wp.tile([128, 4, KC, 128], FP8, tag="w")
            nc.gpsimd.dma_start(out=pw[:], in_=p2w8_d[jt])
            last = jt == JT - 1
            blocks = [(nb * NBS, NBS) for nb in range(NB - 1 if last else NB)]
            if last:
                b0 = (NB - 1) * NBS
                blocks += [(b0, 256), (b0 + 256, 128), (b0 + 384, 64), (b0 + 448, 64)]
            for bi, (col0, ncol) in enumerate(blocks):
                psz = psum.tile([128, NBS], F32, tag="ps", name=f"ps_z{jt}_{bi}")
                gemm_pair(psz, pw[:, 0], (xh8, 0), pw[:, 1], (xh8, 1), col0, ncol)
                psd = psum.tile([128, NBS], F32, tag="ps", name=f"ps_d{jt}_{bi}")
                gemm_pair(psd, pw[:, 2], (xh8, 0), pw[:, 3], rhT8, col0, ncol)
                zt = workp.tile([128, NBS], BF16, tag="act")
                nc.scalar.activation(
                    zt[:, ds(0, ncol)],
                    psz[:, ds(0, ncol)],
                    AFT.Sigmoid,
                    bias=bias_sb[:, jt : jt + 1],
                    scale=INV_SCALE,
                )
                dt_ = workp.tile([128, NBS], BF16, tag="act")
                nc.scalar.activation(
                    dt_[:, ds(0, ncol)],
                    psd[:, ds(0, ncol)],
                    AFT.Tanh,
                    bias=bias_sb[:, 16 + jt : 17 + jt],
                    scale=INV_SCALE,
                )
                zdh = workp.tile([128, NBS], F32, tag="zdh")
                nc.vector.tensor_mul(
                    zdh[:, ds(0, ncol)], zt[:, ds(0, ncol)], dt_[:, ds(0, ncol)]
                )
                # out = (z*dh) * eps + h
                ot = workp.tile([128, NBS], F32, tag="out")
                nc.vector.scalar_tensor_tensor(
                    ot[:, ds(0, ncol)],
                    zdh[:, ds(0, ncol)],
                    float(eps),
                    hTb[:, jt, ds(col0, ncol)],
                    op0=ALU.mult,
                    op1=ALU.add,
                )
                nc.sync.dma_start(
                    out=out_d[ts(jt, 128), ds(col0, ncol)],
                    in_=ot[:, ds(0, ncol)],
                )

    nc.compile()
    return nc


def _get_nc(eps: float):
    key = float(eps)
    if key not in _nc_cache:
        _nc_cache[key] = _build(key)
    return _nc_cache[key]


def _block_weight(wT, dtype, scale=1.0):
    # [1024, 1024] (contraction-major) -> [jt, p, c, j] st. blk[jt,p,c,j] = wT[c*128+p, jt*128+j]
    blk = wT.reshape(KC, 128, JT, 128).transpose(2, 1, 0, 3)
    if scale != 1.0:
        blk = blk * scale
    return np.ascontiguousarray(blk).astype(dtype)


def _block_data(m):
    # per-core [B, 1024] -> [p, c, b] st. blk[p,c,b] = m[b, c*128+p]
    return np.ascontiguousarray(m.T.reshape(KC, 128, B).transpose(1, 0, 2))


def _block_data_db(m):
    # per-core [B, 1024] -> [db, p, c, dbs] column-block-major (1KB rows)
    blk = m.T.reshape(KC, 128, DB, DBS).transpose(2, 1, 0, 3)
    return np.ascontiguousarray(blk)


def _prep_shared(W_z, b_z, U_z, W_r, b_r, U_r, V_h, b_h, W_h):
    F8 = ml_dtypes.float8_e4m3
    A = W_h - W_h.T - GAMMA * np.eye(H, dtype=np.float32)
    blk = lambda w: _block_weight(w.T, F8, SCALE_W)
    # [jt, p, t, kc, 128] packings
    rw8 = np.ascontiguousarray(
        np.stack([blk(W_r), blk(U_r)], axis=2)
    )
    p2w8 = np.ascontiguousarray(
        np.stack([blk(W_z), blk(U_z), blk(V_h), blk(A)], axis=2)
    )
    biases = np.ascontiguousarray(
        np.concatenate(
            [
                b_z.reshape(JT, 128).T,
                b_r.reshape(JT, 128).T,
                b_h.reshape(JT, 128).T,
            ],
            axis=1,
        ).astype(np.float32)
    )
    return {"rw8": rw8, "p2w8": p2w8, "biases": biases}


def _prep_in_maps(x, h_prev, W_z, b_z, U_z, W_r, b_r, U_r, V_h, b_h, W_h):
    BF = ml_dtypes.bfloat16
    F8 = ml_dtypes.float8_e4m3
    h16 = np.asarray(h_prev, np.float32).astype(BF)
    x8 = (np.asarray(x, np.float32) * SCALE_X).astype(F8)
    h8 = (np.asarray(h_prev, np.float32) * SCALE_X).astype(F8)

    shared = _prep_shared(W_z, b_z, U_z, W_r, b_r, U_r, V_h, b_h, W_h)
    in_maps = []
    for c in range(N_CORES):
        sl = slice(c * B, (c + 1) * B)
        # fused [db, kcp, p, (dk,t), dbs]
        st = np.stack([_block_data_db(x8[sl]), _block_data_db(h8[sl])], axis=3)
        st = st.reshape(DB, 128, KC // 2, 2, 2, DBS).transpose(0, 2, 1, 3, 4, 5)
        xh8 = np.ascontiguousarray(st.reshape(DB, KC // 2, 128, 4, DBS))
        in_maps.append({"hT": _block_data(h16[sl]), "xh8": xh8, **shared})
    return in_maps


def run(inputs, trace=False):
    """Returns (full_output [16384,1024] f32, BassKernelResults)."""
    np_in = {k: np.asarray(v, np.float32) for k, v in inputs.items()}
    eps = float(np_in.pop("epsilon"))
    in_maps = _prep_in_maps(**np_in)
    nc = _get_nc(eps)
    res = run_bass_kernel_spmd(
        nc, in_maps, core_ids=list(range(N_CORES)), trace=trace
    )
    out = np.empty((BATCH, H), np.float32)
    for c in range(N_CORES):
        out[c * B : (c + 1) * B, :] = res.results[c]["out"].T
    return out, res


def kernel(**inputs) -> np.ndarray:
    out, _ = run(inputs, trace=False)
    return out
